# revision 30
# baseline (speedup 1.0000x reference)
"""Trainium2 Bass kernel for nn_AttentionMechanism (batched attention with
per-sample queries), data-parallel across 8 NeuronCores.

Math (per batch row b):
    q = msgs @ Wq.T + bq                         [H]
    k_t = Wk @ tau_t + bk ; scores_t = q.k_t/32
    alpha = softmax(scores) ; out = sum_t alpha_t (Wv @ tau_t + bv)

Rewrite used here (exact up to softmax shift invariance):
    qk   = (msgs @ Wq.T + bq) @ Wk               [TAU] per sample
    scores_t = qk . tau_t / 32      (the q.bk term is constant in t -> cancels)
    p_t  = exp(scores_t)            (scores are O(1), no max-subtraction needed)
    ctx  = sum_t p_t tau_t / sum_t p_t
    out  = ctx @ Wv.T + bv          (uses sum alpha = 1)

Implementation notes:
  - the large inputs (traj, msgs, Wq, Wk) are cast to bf16 on the host in
    make_in_maps, halving HBM traffic; internal compute was already bf16
    (rel err stays ~4e-3, tolerance is 2e-2).
  - streaming/aux pools are allocated BEFORE the nested setup pool so the
    traj chunk DMAs do not inherit a dependency on setup via SBUF reuse.
  - setup: WqT and msgsT land via xbar dma_start_transpose (bf16), then
    qT[h,b] = WqT.T @ msgsT directly on the PE; bq folds into the
    PSUM->SBUF copy as a per-partition activation bias; qk accumulates
    j-outer as the Wk chunk DMAs land.
  - scores: per-t TT product with qk, two TT fold passes, then per-t
    reduction split between scalar activation-accum and vector
    tensor_reduce.  (tensor_tensor_reduce would fuse all of this but
    crashes this hardware path.)
  - ctx accumulation multiplies p_t into tau on the TensorEngine: lhsT is
    diag(p_t), built for a whole chunk by ONE vector TT (identity
    broadcast across t, p broadcast across columns); rhs is the raw
    streamed tau chunk.  No DVE 2-port-mode ops ever run, so GPSIMD
    (SWDGE descriptor generation) is never locked out.
"""

import math

import numpy as np

try:
    import ml_dtypes

    BF16_NP = ml_dtypes.bfloat16
except ImportError:  # pragma: no cover
    import jax.numpy as jnp

    BF16_NP = jnp.bfloat16

import concourse.bass as bass
import concourse.bacc as bacc
import concourse.tile as tile
from concourse import mybir
from concourse.bass_utils import run_bass_kernel_spmd
from concourse.masks import make_identity

F32 = mybir.dt.float32
BF16 = mybir.dt.bfloat16

B = 2048
T = 32
TAU = 1024
MSG = 512
HID = 1024
VDIM = 128
N_CORES = 8
B_LOCAL = B // N_CORES

Alu = mybir.AluOpType
Act = mybir.ActivationFunctionType


def build(b_local=B_LOCAL, t_chunk=8, chunk_bufs=5, n_warm_mm=24,
          reduce_scalar_mod=7, diag_engine="gpsimd"):
    assert b_local % 128 == 0 and T % t_chunk == 0
    n_btiles = b_local // 128
    n_chunks = T // t_chunk
    chunk_free = t_chunk * TAU

    nc = bacc.Bacc("TRN2", target_bir_lowering=False, debug=False)

    traj = nc.declare_dram_parameter(
        "imagined_trajectory", [b_local, T * TAU], BF16, isOutput=False
    )
    msgs = nc.declare_dram_parameter(
        "received_messages", [b_local, MSG], BF16, isOutput=False
    )
    Wq = nc.declare_dram_parameter("Wq", [HID, MSG], BF16, isOutput=False)
    bq = nc.declare_dram_parameter("bq", [HID], F32, isOutput=False)
    Wk = nc.declare_dram_parameter("Wk", [HID, TAU], BF16, isOutput=False)
    Wv = nc.declare_dram_parameter("Wv", [VDIM, TAU], F32, isOutput=False)
    bv = nc.declare_dram_parameter("bv", [VDIM], F32, isOutput=False)
    out = nc.declare_dram_parameter("out", [b_local, VDIM], F32, isOutput=True)

    HQ = HID // 128  # 8 h-chunks
    MQ = MSG // 128  # 4 m-chunks
    CQ = TAU // 128  # 8 c-chunks

    with tile.TileContext(nc) as tc:
        with (
            tc.tile_pool(name="const", bufs=1) as const,
            tc.tile_pool(name="persist", bufs=1) as persist,
            tc.tile_pool(name="stream", bufs=chunk_bufs) as stream,
            tc.tile_pool(name="diagp", bufs=2) as diagp,
            tc.tile_pool(name="aux", bufs=2) as aux,
            tc.tile_pool(name="outp", bufs=2) as outp,
            tc.tile_pool(name="psum_tr", bufs=2, space="PSUM") as psum_tr,
        ):
            # ---- Wk first on the SWDGE queue (needed for qk), then traj ----
            # ---- DMA plan: the sync ring carries ONE xbar transpose (WqT,
            # at the head -- each dma_start_transpose is a serialization
            # barrier for the DMA system) followed by the traj chunks; the
            # scalar ring carries msgs/Wk/Wv/bv in parallel; the tiny
            # scattered bq rearrange goes to the idle SWDGE queue.
            msgs_sb = []
            for bi in range(n_btiles):
                ms = const.tile([128, MSG], BF16, tag=f"msgs{bi}", name=f"msgs{bi}")
                nc.sync.dma_start(out=ms, in_=msgs[bi * 128 : (bi + 1) * 128, :])
                msgs_sb.append(ms)
            WqT_b = persist.tile([128, MQ, HID], BF16)  # [m-part, mi, h]
            nc.sync.dma_start_transpose(WqT_b, Wq[:, :])
            Wk_b = persist.tile([128, HQ, TAU], BF16)  # [h-part, h-chunk, c]
            for k in range(4):
                nc.sync.dma_start(
                    out=Wk_b[:, 2 * k : 2 * k + 2, :],
                    in_=Wk[2 * k * 128 : (2 * k + 2) * 128, :].rearrange(
                        "(j p) c -> p j c", p=128
                    ),
                )
            bq_sb = const.tile([128, HQ], F32)
            nc.gpsimd.dma_start(
                out=bq_sb, in_=bq[:].rearrange("(j p) -> p j", p=128)
            )
            bv_sb = const.tile([1, VDIM], F32)
            nc.scalar.dma_start(out=bv_sb, in_=bv[None, :])
            Wv_sb = const.tile([VDIM, TAU], F32)
            nc.scalar.dma_start(out=Wv_sb, in_=Wv[:, :])

            # ---- constants ----
            ident_f = const.tile([128, 128], F32)
            make_identity(nc, ident_f)
            ident_b = const.tile([128, 128], BF16)
            make_identity(nc, ident_b)
            onespad_b = const.tile([128, 128], BF16)
            nc.vector.memset(onespad_b, 0.0)
            nc.vector.memset(onespad_b[0:1, :], 1.0)
            bvpad_b = const.tile([128, VDIM], BF16)
            nc.vector.memset(bvpad_b, 0.0)
            nc.vector.tensor_copy(out=bvpad_b[0:1, :], in_=bv_sb)

            # warm the PE (HAM) while the DMAs run
            for w in range(n_warm_mm):
                pw = psum_tr.tile([128, 128], F32, tag="tr", name="pw")
                nc.tensor.matmul(pw, lhsT=ident_b, rhs=ident_b, start=True, stop=True)

            WvT_b = persist.tile([128, CQ, VDIM], BF16)  # [c-part, c-chunk, d]
            qk_b = [
                persist.tile([128, TAU], BF16, tag=f"qkb{i}", name=f"qkb{i}")
                for i in range(n_btiles)
            ]

            # ---------- setup: qT = WqT.T @ msgsT (+bq), then qk = qT.T @ Wk
            with (
                tc.tile_pool(name="wtmp", bufs=1) as wtmp,
                tc.tile_pool(name="psum_setup", bufs=2, space="PSUM") as psum_setup,
            ):
                # Wv [VDIM=128, TAU] -> WvT blocks [c-part, d]
                for j in range(CQ):
                    pt = psum_tr.tile([128, 128], F32, tag="tr", name="pt")
                    nc.tensor.transpose(pt, Wv_sb[:, j * 128 : (j + 1) * 128], ident_f)
                    nc.vector.tensor_copy(out=WvT_b[:, j, :], in_=pt)

                # msgs -> msgsT blocks [m-part, b] (PE transposes, bf16)
                msgsT_b = wtmp.tile([128, MQ, b_local], BF16)
                for bi in range(n_btiles):
                    for mi in range(MQ):
                        ptm = psum_setup.tile(
                            [128, 128], BF16, tag="qt", name="ptm"
                        )
                        nc.tensor.transpose(
                            ptm, msgs_sb[bi][:, mi * 128 : (mi + 1) * 128], ident_b
                        )
                        nc.vector.tensor_copy(
                            out=msgsT_b[:, mi, bi * 128 : (bi + 1) * 128], in_=ptm
                        )

                # qT[h-block j][h, b] = sum_mi WqT[:,mi,j].T @ msgsT[:,mi,:]
                # bias bq folds into the copy (per-partition ACT bias).
                qT_b = wtmp.tile([128, HQ, b_local], BF16)
                for j in range(HQ):
                    qp = psum_setup.tile(
                        [128, b_local], F32, tag="qt", name=f"qp{j}"
                    )
                    for mi in range(MQ):
                        nc.tensor.matmul(
                            qp,
                            lhsT=WqT_b[:, mi, j * 128 : (j + 1) * 128],
                            rhs=msgsT_b[:, mi, :],
                            start=(mi == 0),
                            stop=(mi == MQ - 1),
                        )
                    nc.scalar.activation(
                        out=qT_b[:, j, :],
                        in_=qp,
                        func=Act.Identity,
                        bias=bq_sb[:, j : j + 1],
                    )

                # qk[b, c] = q @ Wk, j-outer so matmuls chase the Wk DMAs
                pq = [
                    psum_setup.tile([128, TAU], F32, tag="mm", name=f"pq{bi}")
                    for bi in range(n_btiles)
                ]
                for j in range(HQ):
                    for nh in range(2):
                        nsl = slice(nh * 512, (nh + 1) * 512)
                        for bi in range(n_btiles):
                            nc.tensor.matmul(
                                pq[bi][:, nsl],
                                lhsT=qT_b[:, j, bi * 128 : (bi + 1) * 128],
                                rhs=Wk_b[:, j, nsl],
                                start=(j == 0),
                                stop=(j == HQ - 1),
                            )
                for bi in range(n_btiles):
                    nc.scalar.mul(out=qk_b[bi], in_=pq[bi], mul=1.0 / math.sqrt(HID))

            # preload the exp table (after the setup DMA dispatches so the
            # ~2.7us table load does not block DMA issue on the scalar ring)
            exp_warm = const.tile([128, 1], F32)
            nc.vector.memset(exp_warm, 0.0)
            exp_warm2 = const.tile([128, 1], F32)
            nc.scalar.activation(out=exp_warm2, in_=exp_warm, func=Act.Exp)

            # ---------- main loop ----------
            psum_ctx_cm = tc.tile_pool(name="psum_ctx", bufs=2, space="PSUM")
            psum_ctx = psum_ctx_cm.__enter__()
            dump256 = aux.tile([128, 256], BF16, tag="dump256", name="dump256", bufs=1)
            for bi in range(n_btiles):
                bsl = slice(bi * 128, (bi + 1) * 128)
                ctx_ps = psum_ctx.tile([128, TAU], F32, tag="ctxps", name="ctx_ps")
                scores = aux.tile([128, T], F32, tag="scores", name="scores")
                p_b = aux.tile([128, T], BF16, tag="p", name="p_b")

                for ci in range(n_chunks):
                    chunk_bf = stream.tile(
                        [128, t_chunk, TAU], BF16, tag="chunk", name="chunk_bf"
                    )
                    c0 = ci * chunk_free
                    nc.sync.dma_start(
                        out=chunk_bf,
                        in_=traj[bsl, c0 : c0 + chunk_free].rearrange(
                            "p (t c) -> p t c", t=t_chunk
                        ),
                    )

                    # scores[:, col] = sum_c chunk[:, tt, c] * qk[:, c]
                    prod = diagp.tile(
                        [128, t_chunk, TAU], BF16, tag="prod", name="prod"
                    )
                    for tt in range(t_chunk):
                        nc.vector.tensor_tensor(
                            out=prod[:, tt, :],
                            in0=chunk_bf[:, tt, :],
                            in1=qk_b[bi],
                            op=Alu.mult,
                        )
                    fold1 = diagp.tile(
                        [128, t_chunk, 512], BF16, tag="fold1", name="fold1"
                    )
                    nc.vector.tensor_tensor(
                        out=fold1,
                        in0=prod[:, :, 0:512],
                        in1=prod[:, :, 512:1024],
                        op=Alu.add,
                    )
                    fold2 = diagp.tile(
                        [128, t_chunk, 256], BF16, tag="fold2", name="fold2"
                    )
                    nc.vector.tensor_tensor(
                        out=fold2,
                        in0=fold1[:, :, 0:256],
                        in1=fold1[:, :, 256:512],
                        op=Alu.add,
                    )
                    terminal = bi == n_btiles - 1 and ci == n_chunks - 1
                    for tt in range(t_chunk):
                        col = ci * t_chunk + tt
                        if not terminal and col % 8 < reduce_scalar_mod:
                            nc.scalar.activation(
                                out=dump256,
                                in_=fold2[:, tt, :],
                                func=Act.Copy,
                                accum_out=scores[:, col : col + 1],
                            )
                        else:
                            nc.vector.tensor_reduce(
                                out=scores[:, col : col + 1],
                                in_=fold2[:, tt, :],
                                axis=mybir.AxisListType.X,
                                op=Alu.add,
                            )

                    csl = slice(ci * t_chunk, (ci + 1) * t_chunk)
                    nc.scalar.activation(
                        out=p_b[:, csl], in_=scores[:, csl], func=Act.Exp
                    )

                    # diag(p_t) for the whole chunk in ONE op:
                    # ident broadcast over t  X  p broadcast over columns
                    diag_c = diagp.tile(
                        [128, t_chunk, 128], BF16, tag="diag", name="diag_c"
                    )
                    p_sl = p_b[:, csl]
                    ident_rep = bass.AP(
                        tensor=ident_b.tensor,
                        offset=ident_b.offset,
                        ap=[ident_b.ap[0], [0, t_chunk], ident_b.ap[1]],
                    )
                    p_rep = bass.AP(
                        tensor=p_sl.tensor,
                        offset=p_sl.offset,
                        ap=[p_sl.ap[0], [1, t_chunk], [0, 128]],
                    )
                    if diag_engine == "gpsimd":
                        nc.gpsimd.tensor_tensor(
                            out=diag_c, in0=ident_rep, in1=p_rep, op=Alu.mult
                        )
                    else:
                        nc.vector.tensor_tensor(
                            out=diag_c, in0=ident_rep, in1=p_rep, op=Alu.mult
                        )

                    # HAM fillers: cheap matmuls that become runnable mid-gap
                    # (as soon as the vector stages produce their inputs), so
                    # the PE never idles a full MID window and re-throttles.
                    pw1 = psum_tr.tile([128, 128], F32, tag="tr", name="pw1")
                    nc.tensor.matmul(
                        pw1, lhsT=ident_b, rhs=prod[:, 0, 0:128],
                        start=True, stop=True,
                    )
                    pw2 = psum_tr.tile([128, 128], F32, tag="tr", name="pw2")
                    nc.tensor.matmul(
                        pw2, lhsT=ident_b, rhs=fold2[:, 0, 0:128],
                        start=True, stop=True,
                    )
                    for tt in range(t_chunk):
                        first = ci == 0 and tt == 0
                        last = ci == n_chunks - 1 and tt == t_chunk - 1
                        for nh in range(2):
                            nc.tensor.matmul(
                                ctx_ps[:, nh * 512 : (nh + 1) * 512],
                                lhsT=diag_c[:, tt, :],
                                rhs=chunk_bf[:, tt, nh * 512 : (nh + 1) * 512],
                                start=first,
                                stop=last,
                            )

                # normalize, project: out = (ctx / sum p) @ Wv.T + bv
                s_sum = aux.tile([128, 1], F32, tag="ssum", name="s_sum")
                nc.vector.tensor_reduce(
                    out=s_sum, in_=p_b, axis=mybir.AxisListType.X, op=Alu.add
                )
                rinv = aux.tile([128, 1], F32, tag="rinv", name="rinv")
                nc.vector.reciprocal(out=rinv, in_=s_sum)
                ctxn_f = aux.tile([128, TAU], F32, tag="ctxn", name="ctxn_f")
                nc.scalar.activation(
                    out=ctxn_f,
                    in_=ctx_ps,
                    func=Act.Copy,
                    scale=rinv,
                )
                ctxT_b = aux.tile([128, CQ, 128], BF16, tag="ctxT", name="ctxT_b")
                for j in range(CQ):
                    ptb = psum_tr.tile([128, 128], F32, tag="tr", name="ptb")
                    nc.tensor.transpose(
                        ptb, ctxn_f[:, j * 128 : (j + 1) * 128], ident_f
                    )
                    nc.scalar.copy(out=ctxT_b[:, j, :], in_=ptb)
                pm = psum_tr.tile([128, VDIM], F32, tag="tr", name="pm")
                for j in range(CQ):
                    nc.tensor.matmul(
                        pm,
                        lhsT=ctxT_b[:, j, :],
                        rhs=WvT_b[:, j, :],
                        start=(j == 0),
                        stop=False,
                    )
                nc.tensor.matmul(
                    pm,
                    lhsT=onespad_b,
                    rhs=bvpad_b,
                    start=False,
                    stop=True,
                )
                msg_out = outp.tile([128, VDIM], F32, tag="msg", name="msg_out")
                nc.scalar.copy(out=msg_out, in_=pm)
                nc.scalar.dma_start(out=out[bsl, :], in_=msg_out)
            psum_ctx_cm.__exit__(None, None, None)

    nc.compile()
    return nc


_NC_CACHE = {}


def _get_nc():
    key = "default"
    if key not in _NC_CACHE:
        _NC_CACHE[key] = build()
    return _NC_CACHE[key]


def make_in_maps(imagined_trajectory, received_messages, Wq, bq, Wk, Wv, bv):
    bl = B_LOCAL
    Wq_h = np.asarray(Wq, np.float32).astype(BF16_NP)
    Wk_h = np.asarray(Wk, np.float32).astype(BF16_NP)
    msgs_h = np.asarray(received_messages, np.float32).astype(BF16_NP)
    traj_h = np.asarray(imagined_trajectory, np.float32).astype(BF16_NP)
    in_maps = []
    for i in range(N_CORES):
        sl = slice(i * bl, (i + 1) * bl)
        in_maps.append(
            {
                "imagined_trajectory": np.ascontiguousarray(traj_h[sl]),
                "received_messages": np.ascontiguousarray(msgs_h[sl]),
                "Wq": Wq_h,
                "bq": np.asarray(bq, dtype=np.float32),
                "Wk": Wk_h,
                "Wv": np.asarray(Wv, dtype=np.float32),
                "bv": np.asarray(bv, dtype=np.float32),
            }
        )
    return in_maps


def kernel(
    imagined_trajectory,
    received_messages,
    Wq,
    bq,
    Wk,
    bk,
    Wv,
    bv,
):
    nc = _get_nc()
    in_maps = make_in_maps(
        imagined_trajectory, received_messages, Wq, bq, Wk, Wv, bv
    )
    res = run_bass_kernel_spmd(nc, in_maps, list(range(N_CORES)))
    return np.concatenate([res.results[i]["out"] for i in range(N_CORES)], axis=0)
